# revision 2
# baseline (speedup 1.0000x reference)
"""MultiHeadSimilarity kernel for 8 Trainium2 NeuronCores.

Reference computation (per batch b):
    Q = wq @ x[b];  K = wk @ y[b]                       (channel-mixing matmuls)
    per head h (d=64):  A = relu(Qh^T Kh) * scale, masked by xy_mask
    C = A @ Kh^T, normalized per-row by 1/max(sum(mask, y), 1)
    out = wo @ (0.5 * (Q + C))

Sharding: data-parallel over batch; 16 batches / 8 cores = 2 per core.
Weights replicated. No cross-core communication.

Device algorithm (per core, fp16 compute with fp32 PSUM accumulation):
  - The mask row-normalization (1/(8*max(nel,1)), which also folds the
    1/sqrt(d) attention scale) is computed on the HOST and pre-multiplied
    into the transposed mask -> MTI.  The device then computes
    Am = relu(A) * MTI and C comes out of the PE pre-normalized; E is a
    single add of C (PSUM) and Q.  This removes the ones^T@mask row-count
    matmuls, the reciprocal chain and the per-tile E multiply.
  - KT (the K transpose needed as the C-contraction stationary) is made by
    PE transpose instructions from K (4 per y-tile, via an identity), not by
    a second full projection: 4096 instead of 16384 PE columns per batch.
  - A is computed transposed (y on partitions) per head; relu+mask are
    balanced across three engines: DVE scalar_tensor_tensor, or ScalarE
    relu + multiply on DVE or GpSimd.  0.5 is folded into woT on the host.
"""
import sys

if "/opt/trn_rl_repo" not in sys.path:
    sys.path.insert(0, "/opt/trn_rl_repo")

import numpy as np

import concourse.tile as tile
from concourse import bacc, masks, mybir
from concourse.bass_utils import run_bass_kernel_spmd

F16 = mybir.dt.float16
F32 = mybir.dt.float32
AL = mybir.AluOpType
RELU = mybir.ActivationFunctionType.Relu

N_CORES = 8
B, U, LX, LY, H, D = 16, 512, 1024, 1024, 8, 64
BPC = B // N_CORES          # batches per core
KB = U // 128               # 4  k-tiles over channels
HP = H // 2                 # 4  head pairs
YT = LY // 128              # 8  y tiles
XH = LX // 512              # 2  x halves
INV_SCALE = float(D) ** 0.5  # 8.0; attention scale = 1/8 (folded into MTI)

TRACE = False
_CACHE = {}


class Balance:
    """Greedy static load balancer between DVE / ACT / GpSimd."""

    # measured per-op costs (ns) as functions of free-dim length
    @staticmethod
    def dve_psum(fd):          # psum-f32-source op (stt / cast / copy): 1x mode
        return (fd + 130) / 0.96

    @staticmethod
    def dve_f16(fd):           # f16 x f16 -> f16 tensor_tensor: 2x mode
        return (fd / 2 + 130) / 0.96

    @staticmethod
    def act_op(fd):            # scalar-engine activation / copy
        return (fd + 290) / 1.2

    @staticmethod
    def gps_f16(fd):           # gpsimd f16 tensor_tensor (est; calibrated)
        return (fd + 200) / 0.85

    def __init__(self, nc):
        self.nc = nc
        self.t = {"v": 0.0, "s": 0.0, "g": 0.0}

    def copy(self, dst, src, fd):
        """psum -> sbuf copy, DVE or ACT."""
        dve = self.dve_psum(fd)
        act = self.act_op(fd)
        if self.t["v"] + dve <= self.t["s"] + act:
            self.t["v"] += dve
            self.nc.vector.tensor_copy(dst, src)
        else:
            self.t["s"] += act
            self.nc.scalar.copy(dst, src)

    def relu_mask_pair(self, out, a_ps, mtf_b, mti_2d, tmp_pool, name):
        """out[(128,2,512) f16] = relu(a_ps) * MTI broadcast over head dim.

        route 1: one fused DVE scalar_tensor_tensor at FD=1024;
        route 2: ScalarE relu at FD=1024 + DVE f16 2x multiply;
        route 3: ScalarE relu + two GpSimd f16 multiplies (no broadcast AP).
        """
        stt = self.dve_psum(1024)
        act = self.act_op(1024)
        mul_v = self.dve_f16(1024)
        mul_g = 2 * self.gps_f16(512)
        m1 = max(self.t["v"] + stt, self.t["s"], self.t["g"])
        m2 = max(self.t["v"] + mul_v, self.t["s"] + act, self.t["g"])
        m3 = max(self.t["v"], self.t["s"] + act, self.t["g"] + mul_g)
        best = min(m1, m2, m3)
        if best == m1:
            self.t["v"] += stt
            self.nc.vector.scalar_tensor_tensor(out[:], a_ps[:], 0.0, mtf_b,
                                                AL.max, AL.mult)
        else:
            at = tmp_pool.tile([128, 2, 512], F16, tag="at", name=name)
            self.t["s"] += act
            self.nc.scalar.activation(at[:], a_ps[:], RELU)
            if best == m2:
                self.t["v"] += mul_v
                self.nc.vector.tensor_tensor(out[:], at[:], mtf_b, AL.mult)
            else:
                self.t["g"] += mul_g
                for j in range(2):
                    self.nc.gpsimd.tensor_tensor(out[:, j, :], at[:, j, :],
                                                 mti_2d, AL.mult)


def _build():
    nc = bacc.Bacc("TRN2", target_bir_lowering=False, debug=False,
                   num_devices=N_CORES)
    x_e = nc.dram_tensor("x", [BPC, U, LX], F16, kind="ExternalInput")
    y_e = nc.dram_tensor("y", [BPC, U, LY], F16, kind="ExternalInput")
    mt_e = nc.dram_tensor("mt", [BPC, LY, LX], F16, kind="ExternalInput")
    w_all_e = nc.dram_tensor("w_all", [3, U, U], F16, kind="ExternalInput")
    o_e = nc.dram_tensor("o", [BPC, U, LX], F32, kind="ExternalOutput")

    with tile.TileContext(nc) as tc:
        _emit(nc, tc, x_e, y_e, mt_e, w_all_e, o_e)
    nc.compile()
    return nc


def _emit(nc, tc, x_e, y_e, mt_e, w_all_e, o_e):
    import contextlib
    bal = Balance(nc)
    ctx = contextlib.ExitStack()
    with ctx:
        wp = ctx.enter_context(tc.tile_pool(name="wp", bufs=1))
        io = ctx.enter_context(tc.tile_pool(name="io", bufs=2))
        pr = ctx.enter_context(tc.tile_pool(name="pr", bufs=2))
        amp = ctx.enter_context(tc.tile_pool(name="amp", bufs=4))
        osp = ctx.enter_context(tc.tile_pool(name="osp", bufs=2))
        pa = ctx.enter_context(tc.tile_pool(name="pa", bufs=3, space="PSUM"))
        pc = ctx.enter_context(tc.tile_pool(name="pc", bufs=2, space="PSUM"))

        # weights, loaded once (per-k so the first projection can start early)
        WQT = wp.tile([128, KB, U], F16, tag="wqt")
        WKT = wp.tile([128, KB, U], F16, tag="wkt")
        WOT = wp.tile([128, KB, U], F16, tag="wot")
        for wi, w_t in enumerate((WQT, WKT, WOT)):
            for k in range(KB):
                nc.scalar.dma_start(
                    w_t[:, k, :], w_all_e.ap()[wi, k * 128:(k + 1) * 128, :])
        ident = wp.tile([128, 128], F16, tag="ident")
        masks.make_identity(nc, ident[:])

        for b in range(BPC):
            # ---- input loads (split so compute can start early) ----
            X = io.tile([128, KB, LX], F16, tag="x", name=f"x{b}")
            Y = io.tile([128, KB, LY], F16, tag="y", name=f"y{b}")
            for k in range(KB):
                nc.sync.dma_start(X[:, k, :], x_e.ap()[b, k * 128:(k + 1) * 128, :])
            for k in range(KB):
                nc.gpsimd.dma_start(Y[:, k, :], y_e.ap()[b, k * 128:(k + 1) * 128, :])
            bal.t["g"] += 4 * 650
            MTI = io.tile([128, YT, LX], F16, tag="mti", name=f"mti{b}")
            for t in range(YT):
                (nc.sync if t % 2 == 0 else nc.gpsimd).dma_start(
                    MTI[:, t, :], mt_e.ap()[b, t * 128:(t + 1) * 128, :])
            bal.t["g"] += 4 * 650

            # ---- projections: Q = wqT.T @ x, K = wkT.T @ y ----
            Q = pr.tile([128, KB, LX], F16, tag="q", name=f"q{b}")
            K = pr.tile([128, KB, LY], F16, tag="k", name=f"k{b}")
            for w_t, src, dst in ((WQT, X, Q), (WKT, Y, K)):
                for m in range(KB):
                    ps = pa.tile([128, 2, 512], F32, tag="a",
                                 name=f"pj{b}_{dst.name}_{m}")
                    for k in range(KB):
                        for n in range(XH):
                            nc.tensor.matmul(
                                ps[:, n, :], w_t[:, k, m * 128:(m + 1) * 128],
                                src[:, k, n * 512:(n + 1) * 512],
                                start=(k == 0), stop=(k == KB - 1))
                    bal.copy(dst[:, m, :], ps[:], 1024)

            # ---- KT via PE transpose (y on partitions, channels on free) ----
            KT = pr.tile([128, YT, U], F16, tag="kt", name=f"kt{b}")
            for yt in range(YT):
                ktp = pc.tile([128, 4, 128], F16, tag="c", name=f"ktp{b}_{yt}")
                for k in range(KB):
                    nc.tensor.transpose(
                        ktp[:, k, :], K[:, k, yt * 128:(yt + 1) * 128], ident[:])
                bal.copy(KT[:, yt, :], ktp[:], 512)

            # ---- attention ----
            E = pr.tile([128, KB, LX], F16, tag="e", name=f"e{b}")
            for hp in range(HP):
                for xh in range(XH):
                    xs = slice(xh * 512, (xh + 1) * 512)
                    # both heads accumulate into ONE bank: j0 at partitions
                    # 0-63 (col group 0), j1 at 64-127 (col group 64).
                    C = pc.tile([128, 512], F32, tag="c", name=f"c_{b}_{hp}_{xh}")
                    for yt in range(YT):
                        A = pa.tile([128, 2, 512], F32, tag="a",
                                    name=f"a_{b}_{hp}_{xh}_{yt}")
                        for j in range(2):
                            hs = slice(64 * j, 64 * (j + 1))
                            nc.tensor.matmul(
                                A[:, j, :], K[hs, hp, yt * 128:(yt + 1) * 128],
                                Q[hs, hp, xs], start=True, stop=True)
                        Am = amp.tile([128, 2, 512], F16, tag="am", bufs=6,
                                      name=f"am_{b}_{hp}_{xh}_{yt}")
                        mtf_b = MTI[:, yt, xs].unsqueeze(1).broadcast_to((128, 2, 512))
                        bal.relu_mask_pair(Am, A, mtf_b, MTI[:, yt, xs], amp,
                                           f"at_{b}_{hp}_{xh}_{yt}")
                        for j in range(2):
                            nc.tensor.matmul(
                                C[64 * j:64 * (j + 1), :],
                                KT[:, yt, hp * 128 + 64 * j: hp * 128 + 64 * (j + 1)],
                                Am[:, j, :], start=(yt == 0), stop=(yt == YT - 1),
                                skip_group_check=True)
                    # E = Q + C (C is pre-normalized via MTI); DVE only
                    # (ACT has no tensor-tensor, GpSimd has no PSUM access)
                    nc.vector.tensor_tensor(E[:, hp, xs], C[:], Q[:, hp, xs], AL.add)
                    bal.t["v"] += bal.dve_psum(512)

            # ---- output projection ----
            for m in range(KB):
                ps = pa.tile([128, 2, 512], F32, tag="a", name=f"po{b}_{m}")
                for k in range(KB):
                    for n in range(XH):
                        nc.tensor.matmul(ps[:, n, :],
                                         WOT[:, k, m * 128:(m + 1) * 128],
                                         E[:, k, n * 512:(n + 1) * 512],
                                         start=(k == 0), stop=(k == KB - 1))
                oS = osp.tile([128, 2, 512], F32, tag="os", name=f"os{b}_{m}")
                bal.copy(oS[:], ps[:], 1024)
                nc.sync.dma_start(
                    o_e.ap()[b, m * 128:(m + 1) * 128, :].rearrange(
                        "p (a c) -> p a c", a=2),
                    oS[:])


def _get_nc():
    if "nc" not in _CACHE:
        _CACHE["nc"] = _build()
    return _CACHE["nc"]


def kernel(x, y, xy_mask, wq, wk, wo):
    nc = _get_nc()
    xf = x.astype(np.float16)
    yf = y.astype(np.float16)
    # fold the attention scale and the per-row 1/nel normalization into the
    # transposed mask on the host: MTI[y, x] = mask[x, y] / (8 * max(nel_x, 1))
    nel = xy_mask.sum(axis=2, dtype=np.float32)           # (B, Lx)
    inv = 1.0 / (INV_SCALE * np.maximum(nel, 1.0))
    mtt = (xy_mask.transpose(0, 2, 1).astype(np.float32)
           * inv[:, None, :]).astype(np.float16)
    mtt = np.ascontiguousarray(mtt)
    w_all = np.stack([wq.T, wk.T, (0.5 * wo).T]).astype(np.float16)
    w_all = np.ascontiguousarray(w_all)
    in_maps = [
        {"x": xf[c * BPC:(c + 1) * BPC], "y": yf[c * BPC:(c + 1) * BPC],
         "mt": mtt[c * BPC:(c + 1) * BPC], "w_all": w_all}
        for c in range(N_CORES)
    ]
    res = run_bass_kernel_spmd(nc, in_maps, list(range(N_CORES)), trace=TRACE)
    if TRACE:
        _CACHE["last_exec_time_ns"] = res.exec_time_ns
        _CACHE["last_profile_json"] = res.profile_json
    return np.concatenate([res.results[c]["o"] for c in range(N_CORES)], axis=0)


# revision 5
# speedup vs baseline: 1.3526x; 1.3526x over previous
"""MultiHeadSimilarity kernel for 8 Trainium2 NeuronCores.

Reference computation (per batch b):
    Q = wq @ x[b];  K = wk @ y[b]                       (channel-mixing matmuls)
    per head h (d=64):  A = relu(Qh^T Kh) * scale, masked by xy_mask
    C = A @ Kh^T, normalized per-row by 1/max(sum(mask, y), 1)
    out = wo @ (0.5 * (Q + C))

Sharding: data-parallel over batch; 16 batches / 8 cores = 2 per core.
Weights replicated. No cross-core communication.

Device algorithm (per core, fp16 compute with fp32 PSUM accumulation):
  - The mask row-normalization (1/(8*max(nel,1)), which also folds the
    1/sqrt(d) attention scale) is computed on the HOST and pre-multiplied
    into the transposed mask -> MTI.  The device then computes
    Am = relu(A) * MTI and C comes out of the PE pre-normalized; E is a
    single add of C (PSUM) and Q.  This removes the ones^T@mask row-count
    matmuls, the reciprocal chain and the per-tile E multiply.
  - KT (the K transpose needed as the C-contraction stationary) is made by
    PE transpose instructions from K (4 per y-tile, via an identity), not by
    a second full projection: 4096 instead of 16384 PE columns per batch.
  - A is computed transposed (y on partitions) per head; relu+mask are
    balanced across three engines: DVE scalar_tensor_tensor, or ScalarE
    relu + multiply on DVE or GpSimd.  0.5 is folded into woT on the host.
"""
import sys

if "/opt/trn_rl_repo" not in sys.path:
    sys.path.insert(0, "/opt/trn_rl_repo")

import numpy as np

import concourse.tile as tile
from concourse import bacc, masks, mybir
from concourse.bass_utils import run_bass_kernel_spmd

F16 = mybir.dt.float16
F32 = mybir.dt.float32
AL = mybir.AluOpType
RELU = mybir.ActivationFunctionType.Relu

N_CORES = 8
B, U, LX, LY, H, D = 16, 512, 1024, 1024, 8, 64
BPC = B // N_CORES          # batches per core
KB = U // 128               # 4  k-tiles over channels
HP = H // 2                 # 4  head pairs
YT = LY // 128              # 8  y tiles
XH = LX // 512              # 2  x halves
INV_SCALE = float(D) ** 0.5  # 8.0; attention scale = 1/8 (folded into MTI)

TRACE = False
_CACHE = {}


class Balance:
    """Greedy static load balancer between DVE and ACT.

    GpSimd is deliberately NOT used for element-wise work: its throughput is
    ~3x worse than modeled, its queue drowns in semaphore updates, and the
    resulting Am-supply stalls make the PE idle long enough for the HAM
    activity monitor to re-throttle the PE clock to 1.2 GHz (measured: 52%
    of the runtime at K=4/8, a net regression).
    """

    # measured per-op costs (ns) as functions of free-dim length
    @staticmethod
    def dve_psum(fd):          # psum-f32-source op (stt / cast / copy): 1x mode
        return (fd + 130) / 0.96

    @staticmethod
    def dve_f16(fd):           # f16 x f16 -> f16 tensor_tensor: 2x mode
        return (fd / 2 + 130) / 0.96

    @staticmethod
    def act_op(fd):            # scalar-engine activation / copy
        return (fd + 290) / 1.2

    def __init__(self, nc):
        self.nc = nc
        self.t = {"v": 0.0, "s": 0.0}

    def copy(self, dst, src, fd):
        """psum -> sbuf copy, DVE or ACT."""
        dve = self.dve_psum(fd)
        act = self.act_op(fd)
        if self.t["v"] + dve <= self.t["s"] + act:
            self.t["v"] += dve
            self.nc.vector.tensor_copy(dst, src)
        else:
            self.t["s"] += act
            self.nc.scalar.copy(dst, src)

    def relu_mask_pair(self, out, a_ps, mtf_b, tmp_pool, name):
        """out[(128,2,512) f16] = relu(a_ps) * MTI broadcast over head dim.

        route 1: one fused DVE scalar_tensor_tensor at FD=1024;
        route 2: ScalarE relu at FD=1024 + DVE f16 2x multiply.
        """
        stt = self.dve_psum(1024)
        act = self.act_op(1024)
        mul_v = self.dve_f16(1024)
        m1 = max(self.t["v"] + stt, self.t["s"])
        m2 = max(self.t["v"] + mul_v, self.t["s"] + act)
        if m1 <= m2:
            self.t["v"] += stt
            self.nc.vector.scalar_tensor_tensor(out[:], a_ps[:], 0.0, mtf_b,
                                                AL.max, AL.mult)
        else:
            at = tmp_pool.tile([128, 2, 512], F16, tag="at", name=name)
            self.t["s"] += act
            self.nc.scalar.activation(at[:], a_ps[:], RELU)
            self.t["v"] += mul_v
            self.nc.vector.tensor_tensor(out[:], at[:], mtf_b, AL.mult)


def _build():
    nc = bacc.Bacc("TRN2", target_bir_lowering=False, debug=False,
                   num_devices=N_CORES)
    x_e = nc.dram_tensor("x", [BPC, U, LX], F16, kind="ExternalInput")
    y_e = nc.dram_tensor("y", [BPC, U, LY], F16, kind="ExternalInput")
    mt_e = nc.dram_tensor("mt", [BPC, LY, LX], F16, kind="ExternalInput")
    w_all_e = nc.dram_tensor("w_all", [3, U, U], F16, kind="ExternalInput")
    o_e = nc.dram_tensor("o", [BPC, U, LX], F32, kind="ExternalOutput")

    with tile.TileContext(nc) as tc:
        _emit(nc, tc, x_e, y_e, mt_e, w_all_e, o_e)
    nc.compile()
    return nc


def _emit(nc, tc, x_e, y_e, mt_e, w_all_e, o_e):
    import contextlib
    bal = Balance(nc)
    ctx = contextlib.ExitStack()
    with ctx:
        wp = ctx.enter_context(tc.tile_pool(name="wp", bufs=1))
        io = ctx.enter_context(tc.tile_pool(name="io", bufs=2))
        pr = ctx.enter_context(tc.tile_pool(name="pr", bufs=2))
        amp = ctx.enter_context(tc.tile_pool(name="amp", bufs=4))
        osp = ctx.enter_context(tc.tile_pool(name="osp", bufs=2))
        pa = ctx.enter_context(tc.tile_pool(name="pa", bufs=3, space="PSUM"))
        pc = ctx.enter_context(tc.tile_pool(name="pc", bufs=2, space="PSUM"))

        # weights, loaded once (per-k so the first projection can start early)
        WQT = wp.tile([128, KB, U], F16, tag="wqt")
        WKT = wp.tile([128, KB, U], F16, tag="wkt")
        WOT = wp.tile([128, KB, U], F16, tag="wot")
        for wi, w_t in enumerate((WQT, WKT, WOT)):
            for k in range(KB):
                nc.scalar.dma_start(
                    w_t[:, k, :], w_all_e.ap()[wi, k * 128:(k + 1) * 128, :])
        ident = wp.tile([128, 128], F16, tag="ident")
        masks.make_identity(nc, ident[:])

        for b in range(BPC):
            # ---- input loads (split so compute can start early) ----
            X = io.tile([128, KB, LX], F16, tag="x", name=f"x{b}")
            Y = io.tile([128, KB, LY], F16, tag="y", name=f"y{b}")
            for k in range(KB):
                nc.sync.dma_start(X[:, k, :], x_e.ap()[b, k * 128:(k + 1) * 128, :])
            for k in range(KB):
                nc.gpsimd.dma_start(Y[:, k, :], y_e.ap()[b, k * 128:(k + 1) * 128, :])
            MTI = io.tile([128, YT, LX], F16, tag="mti", name=f"mti{b}")
            for t in range(YT):
                (nc.sync if t % 2 == 0 else nc.gpsimd).dma_start(
                    MTI[:, t, :], mt_e.ap()[b, t * 128:(t + 1) * 128, :])

            # ---- projections: Q = wqT.T @ x, K = wkT.T @ y ----
            Q = pr.tile([128, KB, LX], F16, tag="q", name=f"q{b}")
            K = pr.tile([128, KB, LY], F16, tag="k", name=f"k{b}")
            for w_t, src, dst in ((WQT, X, Q), (WKT, Y, K)):
                for m in range(KB):
                    ps = pa.tile([128, 2, 512], F32, tag="a",
                                 name=f"pj{b}_{dst.name}_{m}")
                    for k in range(KB):
                        for n in range(XH):
                            nc.tensor.matmul(
                                ps[:, n, :], w_t[:, k, m * 128:(m + 1) * 128],
                                src[:, k, n * 512:(n + 1) * 512],
                                start=(k == 0), stop=(k == KB - 1))
                    bal.copy(dst[:, m, :], ps[:], 1024)

            # ---- KT via PE transpose (y on partitions, channels on free) ----
            KT = pr.tile([128, YT, U], F16, tag="kt", name=f"kt{b}")
            for yt in range(YT):
                ktp = pc.tile([128, 4, 128], F16, tag="c", name=f"ktp{b}_{yt}")
                for k in range(KB):
                    nc.tensor.transpose(
                        ktp[:, k, :], K[:, k, yt * 128:(yt + 1) * 128], ident[:])
                bal.copy(KT[:, yt, :], ktp[:], 512)

            # ---- attention ----
            # Emission is software-pipelined at depth 2: the PE queue is
            # in-order, so a C-matmul emitted right after its A-matmul would
            # head-block the queue for the full relu+mask latency.  Emitting
            # A(yt+2) before C(yt) keeps the PE streaming (3 A tiles live =
            # pa bufs).  HAM re-throttles the PE clock after ~3.4us of
            # accumulated idle, so PE stalls cost double.
            E = pr.tile([128, KB, LX], F16, tag="e", name=f"e{b}")
            DEPTH = 2
            for hp in range(HP):
                for xh in range(XH):
                    xs = slice(xh * 512, (xh + 1) * 512)
                    # both heads accumulate into ONE bank: j0 at partitions
                    # 0-63 (col group 0), j1 at 64-127 (col group 64).
                    C = pc.tile([128, 512], F32, tag="c", name=f"c_{b}_{hp}_{xh}")
                    ams = [None] * YT

                    def emit_a(yt, hp=hp, xh=xh, xs=xs, C=C, ams=ams):
                        A = pa.tile([128, 2, 512], F32, tag="a",
                                    name=f"a_{b}_{hp}_{xh}_{yt}")
                        for j in range(2):
                            hs = slice(64 * j, 64 * (j + 1))
                            nc.tensor.matmul(
                                A[:, j, :], K[hs, hp, yt * 128:(yt + 1) * 128],
                                Q[hs, hp, xs], start=True, stop=True)
                        Am = amp.tile([128, 2, 512], F16, tag="am", bufs=6,
                                      name=f"am_{b}_{hp}_{xh}_{yt}")
                        mtf_b = MTI[:, yt, xs].unsqueeze(1).broadcast_to((128, 2, 512))
                        bal.relu_mask_pair(Am, A, mtf_b, amp,
                                           f"at_{b}_{hp}_{xh}_{yt}")
                        ams[yt] = Am

                    def emit_c(yt, hp=hp, C=C, ams=ams):
                        for j in range(2):
                            nc.tensor.matmul(
                                C[64 * j:64 * (j + 1), :],
                                KT[:, yt, hp * 128 + 64 * j: hp * 128 + 64 * (j + 1)],
                                ams[yt][:, j, :], start=(yt == 0),
                                stop=(yt == YT - 1), skip_group_check=True)

                    for yt in range(YT):
                        emit_a(yt)
                        if yt >= DEPTH:
                            emit_c(yt - DEPTH)
                    for yt in range(YT - DEPTH, YT):
                        emit_c(yt)
                    # E = Q + C (C is pre-normalized via MTI); DVE only
                    # (ACT has no tensor-tensor, GpSimd has no PSUM access)
                    nc.vector.tensor_tensor(E[:, hp, xs], C[:], Q[:, hp, xs], AL.add)
                    bal.t["v"] += bal.dve_psum(512)

            # ---- output projection ----
            for m in range(KB):
                ps = pa.tile([128, 2, 512], F32, tag="a", name=f"po{b}_{m}")
                for k in range(KB):
                    for n in range(XH):
                        nc.tensor.matmul(ps[:, n, :],
                                         WOT[:, k, m * 128:(m + 1) * 128],
                                         E[:, k, n * 512:(n + 1) * 512],
                                         start=(k == 0), stop=(k == KB - 1))
                oS = osp.tile([128, 2, 512], F32, tag="os", name=f"os{b}_{m}")
                bal.copy(oS[:], ps[:], 1024)
                nc.sync.dma_start(
                    o_e.ap()[b, m * 128:(m + 1) * 128, :].rearrange(
                        "p (a c) -> p a c", a=2),
                    oS[:])


def _get_nc():
    if "nc" not in _CACHE:
        _CACHE["nc"] = _build()
    return _CACHE["nc"]


def kernel(x, y, xy_mask, wq, wk, wo):
    nc = _get_nc()
    xf = x.astype(np.float16)
    yf = y.astype(np.float16)
    # fold the attention scale and the per-row 1/nel normalization into the
    # transposed mask on the host: MTI[y, x] = mask[x, y] / (8 * max(nel_x, 1))
    nel = xy_mask.sum(axis=2, dtype=np.float32)           # (B, Lx)
    inv = 1.0 / (INV_SCALE * np.maximum(nel, 1.0))
    mtt = (xy_mask.transpose(0, 2, 1).astype(np.float32)
           * inv[:, None, :]).astype(np.float16)
    mtt = np.ascontiguousarray(mtt)
    w_all = np.stack([wq.T, wk.T, (0.5 * wo).T]).astype(np.float16)
    w_all = np.ascontiguousarray(w_all)
    in_maps = [
        {"x": xf[c * BPC:(c + 1) * BPC], "y": yf[c * BPC:(c + 1) * BPC],
         "mt": mtt[c * BPC:(c + 1) * BPC], "w_all": w_all}
        for c in range(N_CORES)
    ]
    res = run_bass_kernel_spmd(nc, in_maps, list(range(N_CORES)), trace=TRACE)
    if TRACE:
        _CACHE["last_exec_time_ns"] = res.exec_time_ns
        _CACHE["last_profile_json"] = res.profile_json
    return np.concatenate([res.results[c]["o"] for c in range(N_CORES)], axis=0)
